# revision 60
# baseline (speedup 1.0000x reference)
"""Cross-attention (GQA + RoPE) Trainium2 Bass kernel.

Sharding: 8 cores = 4 batches x 2 head-groups.
  core i -> batch b = i // 2, head-group g = i % 2
  Each core computes 8 query heads / 2 kv heads of one batch and a
  row-parallel partial of the output projection; the host sums the two
  partials per batch.

Key optimizations over the v1 baseline:
  * kv compaction: ~50% of kv positions are masked out; the host gathers
    valid positions (and their RoPE phase tables) and pads to a multiple
    of 128 (TKV_C).  Scores / exp / PV / KV-projection all shrink.
  * All inputs are host-pre-arranged into the exact SBUF layout
    [128, c, X] so every DMA moves large contiguous per-partition rows;
    two DMA queues run in parallel (kv-side on gpsimd, q-side on vector).
  * bf16 inputs and intermediates (PSUM stays f32).
  * reciprocal_approx_fast on a [2, t2] packed denominator pair, one
    K=2 broadcast matmul per head-pair (inv0 -> psum rows 0-63,
    inv1 -> rows 64-127).
  * Software pipelining: the next block's Q-projection pairs and the
    previous block's output-projection slices are interleaved between
    attention head-pairs, so the PE never sits in a dedicated
    projection phase; output-projection PSUM->SBUF copies run on the
    Scalar engine.
"""

import math
from contextlib import ExitStack

import numpy as np
import ml_dtypes

import concourse.bass as bass
import concourse.bacc as bacc
import concourse.mybir as mybir
import concourse.tile as tile
from concourse.bass_utils import run_bass_kernel_spmd

F32 = mybir.dt.float32
R32 = mybir.dt.float32r
BF16 = mybir.dt.bfloat16

D_MODEL = 1024
N_HEADS = 16
NUM_KV_HEADS = 4
D_K = 64
ROPE_BASE = 10000.0
TQ = 2048
N_CORES = 8

NEG_BIAS = -30000.0


def _ktiles(tkv_c):
    """K-projection column tiles: two 256-wide leading tiles (so the first
    projection starts as early as possible behind the kv DMA), 512 after."""
    ks = []
    c0 = 0
    while c0 < tkv_c:
        w = min(256 if c0 < 512 else 512, tkv_c - c0)
        ks.append((c0, w))
        c0 += w
    return ks


def build_bass(tq=TQ, tkv_c=1152, t2=1024):
    """Build the single-core SPMD program (same program on all 8 cores)."""
    nc = bacc.Bacc("TRN2", target_bir_lowering=False, debug=False)
    P = 128
    NCH = tkv_c // 128        # attention kv chunks
    NT2 = tq // t2            # tq blocks
    NHALF = t2 // 512         # 512-wide matmul slices per tq block
    NPAIR = 4                 # head-pair tiles per core
    NSLICE = t2 // 128        # output rows per block
    ktiles = _ktiles(tkv_c)

    NQIN = tq // 1024  # q DMA granularity stays 1024 columns
    q_in = [
        nc.dram_tensor(f"q{i}", [P, 8, 1024], BF16, kind="ExternalInput").ap()
        for i in range(NQIN)
    ]
    kv_in = [
        nc.dram_tensor(f"kv_t{i}", [P, 8, w], BF16, kind="ExternalInput").ap()
        for i, (_c0, w) in enumerate(ktiles)
    ]
    wq = nc.dram_tensor("wq", [P, 8, 512], BF16, kind="ExternalInput").ap()
    wk = nc.dram_tensor("wk", [P, 8, 128], BF16, kind="ExternalInput").ap()
    wv = nc.dram_tensor("wv", [P, 8, 128], BF16, kind="ExternalInput").ap()
    wout = nc.dram_tensor("wout", [P, 4, D_MODEL], BF16, kind="ExternalInput").ap()
    cosK = nc.dram_tensor("cosK", [P, tkv_c], BF16, kind="ExternalInput").ap()
    sinK = nc.dram_tensor("sinK", [P, tkv_c], BF16, kind="ExternalInput").ap()
    cosQ = nc.dram_tensor("cosQ", [P, tq], BF16, kind="ExternalInput").ap()
    sinQ = nc.dram_tensor("sinQ", [P, tq], BF16, kind="ExternalInput").ap()
    maskb = nc.dram_tensor("maskb", [P, NCH], F32, kind="ExternalInput").ap()
    e2 = nc.dram_tensor("e2", [64, P], R32, kind="ExternalInput").ap()
    out = nc.dram_tensor("out", [tq, D_MODEL], F32, kind="ExternalOutput").ap()

    with tile.TileContext(nc) as tc, ExitStack() as ctx:
        const = ctx.enter_context(tc.tile_pool(name="const", bufs=1))
        qpool = ctx.enter_context(tc.tile_pool(name="qpool", bufs=1))
        apool = ctx.enter_context(tc.tile_pool(name="apool", bufs=1))
        workp = ctx.enter_context(tc.tile_pool(name="workp", bufs=3))
        ropep = ctx.enter_context(tc.tile_pool(name="ropep", bufs=2))
        big_bufs = 5 if t2 <= 512 else 2
        acc_bufs = 3 if t2 <= 512 else 2
        pp_big = ctx.enter_context(
            tc.tile_pool(name="pp_big", bufs=big_bufs, space="PSUM")
        )
        pp_acc = ctx.enter_context(
            tc.tile_pool(name="pp_acc", bufs=acc_bufs, space="PSUM")
        )

        def MM(out_ap, lhsT, rhs, start, stop, chain=None):
            inst = nc.tensor.matmul(out_ap, lhsT, rhs, start=start, stop=stop)
            if chain is not None:
                tc.chain_iter_dep(chain, inst.ins)
            return inst

        def chain_dve(inst):
            tc.chain_iter_dep("dve_norm", inst.ins)
            return inst

        # ---- constants set up on-engine (no DMA) --------------------------
        Vt = [const.tile([P, NCH * 65], BF16, name=f"Vt{i}") for i in range(2)]
        for i in range(2):
            nc.gpsimd.memset(
                Vt[i].rearrange("p (c k) -> p c k", k=65)[:, :, 64], 1.0
            )

        # ---- kv-side inputs on the gpsimd queue (critical first) ----------
        wk_sb = const.tile([P, 8, 128], BF16)
        nc.gpsimd.dma_start(out=wk_sb, in_=wk)
        wv_sb = const.tile([P, 8, 128], BF16)
        nc.gpsimd.dma_start(out=wv_sb, in_=wv)
        kv_sb = [
            const.tile([P, 8, w], BF16, name=f"kvt{i}")
            for i, (_c0, w) in enumerate(ktiles)
        ]
        nc.gpsimd.dma_start(out=kv_sb[0], in_=kv_in[0])
        cosK_sb = const.tile([P, tkv_c], BF16)
        nc.gpsimd.dma_start(out=cosK_sb, in_=cosK)
        sinK_sb = const.tile([P, tkv_c], BF16)
        nc.gpsimd.dma_start(out=sinK_sb, in_=sinK)
        mask_sb = const.tile([P, NCH], F32)
        nc.gpsimd.dma_start(out=mask_sb, in_=maskb)
        for i in range(1, len(ktiles)):
            nc.gpsimd.dma_start(out=kv_sb[i], in_=kv_in[i])

        # ---- q-side inputs on the scalar queue ----------------------------
        e2_sb = const.tile([64, P], R32)
        nc.scalar.dma_start(out=e2_sb, in_=e2)
        # inv broadcast staging: head0 inv in row 0, head1 inv in row 32,
        # all other rows memset to a safe finite value (multiplied by e2=0).
        invp_tiles = [const.tile([64, t2], R32, name=f"invp{i}") for i in range(4)]
        for tl in invp_tiles:
            nc.gpsimd.memset(tl.bitcast(F32), 1.0)
        wq_sb = const.tile([P, 8, 512], BF16)
        nc.scalar.dma_start(out=wq_sb, in_=wq)
        q_sb = [const.tile([P, 8, 1024], BF16, name=f"qsb{i}") for i in range(NQIN)]
        nc.scalar.dma_start(out=q_sb[0], in_=q_in[0])
        cosQ_sb = const.tile([P, tq], BF16)
        nc.scalar.dma_start(out=cosQ_sb, in_=cosQ)
        sinQ_sb = const.tile([P, tq], BF16)
        nc.scalar.dma_start(out=sinQ_sb, in_=sinQ)
        # late-needed bulk goes on the gpsimd queue tail to balance the two
        wout_sb = const.tile([P, 4, D_MODEL], BF16)
        nc.gpsimd.dma_start(out=wout_sb, in_=wout)
        for i in range(1, NQIN):
            nc.gpsimd.dma_start(out=q_sb[i], in_=q_in[i])

        Kt = const.tile([P, tkv_c], BF16)

        def rope_apply(dest, ps, col0, width, cos_sb, sin_sb):
            """dest[128, width] (SBUF) = rope(ps[128, width] PSUM), positions
            col0..col0+width. Rows are two stacked heads, each [x1(32); x2(32)]."""
            cs = cos_sb[:, col0 : col0 + width]
            t_cos = ropep.tile([P, t2], F32, tag="rope", name="t_cos")
            t_u = ropep.tile([P, t2], F32, tag="rope", name="t_u")
            tc_ = t_cos[:, :width]
            tu_ = t_u[:, :width]
            nc.vector.tensor_mul(tc_, ps, cs)
            for b0 in (0, 64):
                # sin rows [b0:b0+32] = -sin, [b0+32:b0+64] = +sin
                nc.vector.tensor_mul(
                    tu_[b0 : b0 + 32, :],
                    ps[b0 + 32 : b0 + 64, :],
                    sin_sb[b0 : b0 + 32, col0 : col0 + width],
                )
                nc.vector.tensor_mul(
                    tu_[b0 + 32 : b0 + 64, :],
                    ps[b0 : b0 + 32, :],
                    sin_sb[b0 + 32 : b0 + 64, col0 : col0 + width],
                )
            with nc.allow_low_precision("rope output bf16"):
                nc.vector.tensor_add(dest, tc_, tu_)

        # ---- phase KV: K/V projections (invoked from the pipeline) --------
        def kv_tile(kt):
            kc0, kw = ktiles[kt]
            cols = slice(kc0, kc0 + kw)
            ps_k = pp_big.tile([P, 512], F32, tag="big", name="ps_k")
            pk = ps_k[:, :kw]
            for d in range(8):
                MM(pk, wk_sb[:, d, :], kv_sb[kt][:, d, :], d == 0, d == 7)
            rope_apply(Kt[:, cols], pk, kc0, kw, cosK_sb, sinK_sb)
            for s in range(kw // 128):
                ps_v = pp_big.tile([P, 512], F32, tag="big", name="ps_v")
                pv = ps_v[:, 0:128]
                lv = slice(s * 128, (s + 1) * 128)
                for d in range(8):
                    MM(pv, kv_sb[kt][:, d, lv], wv_sb[:, d, :], d == 0, d == 7)
                c = kc0 // 128 + s
                with nc.allow_low_precision("V bf16"):
                    nc.vector.tensor_copy(
                        out=Vt[0][:, c * 65 : c * 65 + 64], in_=pv[:, 0:64]
                    )
                    nc.vector.tensor_copy(
                        out=Vt[1][:, c * 65 : c * 65 + 64], in_=pv[:, 64:128]
                    )

        # ---- double-generation Qt / attnT tiles ---------------------------
        Qt = [
            [
                qpool.tile([P, t2], BF16, tag=f"Q{j}g{ggen}", name=f"Qt{j}g{ggen}")
                for j in range(NPAIR)
            ]
            for ggen in range(2)
        ]
        At = [
            [
                apool.tile([P, t2], BF16, tag=f"A{j}g{ggen}", name=f"At{j}g{ggen}")
                for j in range(NPAIR)
            ]
            for ggen in range(2)
        ]
        pending = []
        pair_seq = [0]

        def qproj_pair(it2, j):
            ps_q = pp_big.tile([P, t2], F32, tag="big", name="ps_q")
            for half in range(NHALF):
                c0 = it2 * t2 + half * 512
                for d in range(8):
                    MM(
                        ps_q[:, half * 512 : (half + 1) * 512],
                        wq_sb[:, d, j * 128 : (j + 1) * 128],
                        q_sb[c0 // 1024][:, d, c0 % 1024 : c0 % 1024 + 512],
                        d == 0,
                        d == 7,
                    )
            rope_apply(Qt[it2 % 2][j], ps_q, it2 * t2, t2, cosQ_sb, sinQ_sb)

        def flush_norm():
            if not pending:
                return
            U0, U1, invp, j_, attn_cur = pending.pop(0)
            Us = (U0, U1)
            for half in range(NHALF):
                hs = slice(half * 512, (half + 1) * 512)
                ps_b = pp_big.tile([P, 512], F32, tag="big", name="ps_b")
                MM(ps_b, e2_sb, invp[:, hs], True, True, chain="pe_attn")
                for ab, base in ((0, 0), (1, 64)):
                    with nc.allow_low_precision("attnT bf16"):
                        chain_dve(
                            nc.vector.tensor_mul(
                                attn_cur[j_][base : base + 64, hs],
                                Us[ab][0:64, hs],
                                ps_b[base : base + 64, :],
                            )
                        )

        def outproj_slices(it2, slices):
            attn_cur = At[it2 % 2]
            for s in slices:
                ob = ropep.tile([P, D_MODEL], F32, tag="ob", name="ob", bufs=2)
                for n in range(2):
                    ps_f = pp_big.tile([P, 512], F32, tag="big", name="ps_f")
                    for p_ in range(NPAIR):
                        MM(
                            ps_f,
                            attn_cur[p_][:, s * 128 : (s + 1) * 128],
                            wout_sb[:, p_, n * 512 : (n + 1) * 512],
                            p_ == 0,
                            p_ == NPAIR - 1,
                        )
                    nc.scalar.copy(out=ob[:, n * 512 : (n + 1) * 512], in_=ps_f)
                r0 = it2 * t2 + s * 128
                eng = nc.sync if s % 2 == 0 else nc.gpsimd
                eng.dma_start(out=out[r0 : r0 + 128, :], in_=ob)

        def make_pair(it2, j):
            """Resumable attention for head-pair j of block it2: run(c_lo,
            c_hi) emits chunk work; finish() emits the PV tail and the
            normalization prologue (U/den copies, reciprocal, inv pack)."""
            Qt_cur = Qt[it2 % 2]
            attn_cur = At[it2 % 2]
            heads = [(j, 0, 0), (j + 4, 1, 64)]  # (head, kvh, base)
            ps_os = [
                pp_acc.tile([65, t2], F32, tag="acc", name=f"ps_o{ab}")
                for ab in range(2)
            ]
            st = {"prev": None}

            def emit_pv(c_, exs_):
                for ab, (_h, kvh, _base) in enumerate(heads):
                    for half in range(NHALF):
                        MM(
                            ps_os[ab][:, half * 512 : (half + 1) * 512],
                            Vt[kvh][:, c_ * 65 : c_ * 65 + 65],
                            exs_[ab][:, half * 512 : (half + 1) * 512],
                            c_ == 0,
                            c_ == NCH - 1,
                            chain="pe_attn",
                        )

            def run(c_lo, c_hi):
                # PV lags the scores by one chunk so no PE instruction ever
                # reaches the queue head with an unresolved wait.
                for c in range(c_lo, c_hi):
                    exs = []
                    for ab, (_h, kvh, base) in enumerate(heads):
                        ps_s = pp_big.tile([P, t2], F32, tag="big", name="ps_s")
                        for half in range(NHALF):
                            MM(
                                ps_s[:, half * 512 : (half + 1) * 512],
                                Kt[base : base + 64, c * 128 : (c + 1) * 128],
                                Qt_cur[j][base : base + 64, half * 512 : (half + 1) * 512],
                                True,
                                True,
                                chain="pe_attn",
                            )
                        ex = workp.tile([P, t2], BF16, tag="expT", name="ex", bufs=8)
                        nc.scalar.activation(
                            out=ex,
                            in_=ps_s,
                            func=mybir.ActivationFunctionType.Exp,
                            bias=mask_sb[:, c : c + 1],
                            scale=0.125,
                        )
                        exs.append(ex)
                    if st["prev"] is not None:
                        emit_pv(c - 1, st["prev"])
                    st["prev"] = exs

            def finish():
                emit_pv(NCH - 1, st["prev"])
                # flush the previous pair first: its bcast matmul runs now
                # (reciprocal long done), and its muls free ps_b slots early.
                while pending:
                    flush_norm()
                # accumulator copies BEFORE the reciprocal: the in-order DVE
                # must release both PSUM slots promptly.
                invp = invp_tiles[pair_seq[0] % 4]
                pair_seq[0] += 1
                Us = []
                for ab in range(2):
                    U = workp.tile([64, t2], F32, tag="unorm", name="U", bufs=4)
                    chain_dve(nc.vector.tensor_copy(out=U, in_=ps_os[ab][0:64, :]))
                    den = workp.tile([1, t2], F32, tag="den", name="den", bufs=4)
                    chain_dve(
                        nc.vector.tensor_copy(out=den, in_=ps_os[ab][64:65, :])
                    )
                    inv_f = workp.tile([1, t2], F32, tag="invf", name="inv_f", bufs=4)
                    chain_dve(nc.vector.reciprocal_approx_fast(out=inv_f, in_=den))
                    with nc.allow_low_precision("f32r softmax denom"):
                        chain_dve(
                            nc.vector.tensor_copy(
                                out=invp[32 * ab : 32 * ab + 1, :], in_=inv_f
                            )
                        )
                    Us.append(U)
                pending.append((Us[0], Us[1], invp, j, attn_cur))

            return run, finish

        # ---- pipeline -----------------------------------------------------
        qtasks = [(b2, jj) for b2 in range(NT2) for jj in range(NPAIR)]

        def pop_q():
            if qtasks:
                qproj_pair(*qtasks.pop(0))

        # Block-0 pair-0 attention streams directly behind the KV tiles so
        # scoring starts as soon as the first kv tile + Q pair are ready.
        ranges = []
        acc0 = 0
        for _kc0, kw in ktiles:
            ranges.append((acc0, acc0 + kw // 128))
            acc0 += kw // 128
        for kt in range(len(ktiles)):
            kv_tile(kt)
            if kt < 2:
                pop_q()
        # one Q-projection task and (for it2 > 0) the previous block's
        # output slices per attention pair end.
        for it2 in range(NT2):
            nsl = 0
            for j in range(NPAIR):
                run, fin = make_pair(it2, j)
                run(0, NCH)
                fin()
                pop_q()
                if it2 > 0:
                    take = NSLICE * (j + 1) // NPAIR
                    outproj_slices(it2 - 1, range(nsl, take))
                    nsl = take
        while pending:
            flush_norm()
        outproj_slices(NT2 - 1, range(NSLICE))

    nc.compile()
    return nc


# ---------------------------------------------------------------------------
# host-side sharding / prep
# ---------------------------------------------------------------------------

_HEAD_PERM = [0, 4, 1, 5, 2, 6, 3, 7]  # local head order inside pair tiles


def _rope_tables(positions):
    """cos/sin tables [128, len(positions)] with the sign pattern baked in."""
    theta = ROPE_BASE ** (-np.arange(0, D_K, 2, dtype=np.float64) / D_K)  # [32]
    ang = positions.astype(np.float64)[:, None] * theta[None, :]  # [T,32]
    c = np.cos(ang).T.astype(np.float32)  # [32, T]
    s = np.sin(ang).T.astype(np.float32)
    cosF = np.concatenate([c, c, c, c], axis=0)
    sinF = np.concatenate([-s, s, -s, s], axis=0)
    return np.ascontiguousarray(cosF), np.ascontiguousarray(sinF)


def _pack8(a, c, width):
    """[c*128, width] -> [128, c, width] with row d = c_idx*128 + p."""
    return np.ascontiguousarray(a.reshape(c, 128, width).transpose(1, 0, 2))


def make_in_maps(query, key_value, kv_mask, w_q, w_k, w_v, w_out, tq=TQ):
    nb = query.shape[0]
    bf = ml_dtypes.bfloat16

    idxs = [np.nonzero(kv_mask[b])[0] for b in range(nb)]
    nmax = max((len(i) for i in idxs), default=1)
    tkv_c = max(256, int(math.ceil(max(nmax, 1) / 128.0)) * 128)
    nch = tkv_c // 128

    cosQ, sinQ = _rope_tables(np.arange(tq))
    cosQ_bf = cosQ.astype(bf)
    sinQ_bf = sinQ.astype(bf)
    e2 = np.zeros((64, 128), np.float32)
    e2[0, 0:64] = 1.0
    e2[32, 64:128] = 1.0

    col_perm = np.concatenate(
        [np.arange(h * D_K, (h + 1) * D_K) for h in _HEAD_PERM]
    )
    in_maps = []
    for core in range(2 * nb):
        b = core // 2
        g = core % 2
        idx = idxs[b]
        nv = len(idx)

        kv_c = np.zeros((tkv_c, D_MODEL), np.float32)
        kv_c[:nv] = key_value[b][idx]
        kvT = np.ascontiguousarray(kv_c.T)  # [1024, tkv_c]

        pos = np.zeros(tkv_c, np.int64)
        pos[:nv] = idx
        cosK, sinK = _rope_tables(pos)

        maskb = np.full(tkv_c, NEG_BIAS, np.float32)
        maskb[:nv] = 0.0
        maskb = np.ascontiguousarray(maskb.reshape(nch, 128).T)

        qT = np.ascontiguousarray(query[b].T)  # [1024, tq]

        wq_g = w_q[:, g * 512 : (g + 1) * 512][:, col_perm]
        wk_g = w_k[:, g * 128 : (g + 1) * 128]
        wv_g = w_v[:, g * 128 : (g + 1) * 128]
        wout_g = w_out[g * 512 : (g + 1) * 512, :][col_perm, :]

        m = {
            "wq": _pack8(np.ascontiguousarray(wq_g), 8, 512).astype(bf),
            "wk": _pack8(np.ascontiguousarray(wk_g), 8, 128).astype(bf),
            "wv": _pack8(np.ascontiguousarray(wv_g), 8, 128).astype(bf),
            "wout": _pack8(np.ascontiguousarray(wout_g), 4, D_MODEL).astype(bf),
            "cosK": cosK.astype(bf),
            "sinK": sinK.astype(bf),
            "cosQ": cosQ_bf,
            "sinQ": sinQ_bf,
            "maskb": maskb,
            "e2": e2,
        }
        for i, (c0, w) in enumerate(_ktiles(tkv_c)):
            m[f"kv_t{i}"] = _pack8(
                np.ascontiguousarray(kvT[:, c0 : c0 + w]), 8, w
            ).astype(bf)
        for i in range(tq // 1024):
            m[f"q{i}"] = _pack8(
                np.ascontiguousarray(qT[:, i * 1024 : (i + 1) * 1024]), 8, 1024
            ).astype(bf)
        in_maps.append(m)
    return in_maps, tkv_c


_NC_CACHE = {}


T2 = 512


def _get_nc(tq, tkv_c):
    key = (tq, tkv_c, T2)
    if key not in _NC_CACHE:
        _NC_CACHE[key] = build_bass(tq, tkv_c, T2)
    return _NC_CACHE[key]


def _run(inputs, trace=False):
    query = np.asarray(inputs["query"], dtype=np.float32)
    key_value = np.asarray(inputs["key_value"], dtype=np.float32)
    kv_mask = np.asarray(inputs["kv_mask"])
    w_q = np.asarray(inputs["w_q"], dtype=np.float32)
    w_k = np.asarray(inputs["w_k"], dtype=np.float32)
    w_v = np.asarray(inputs["w_v"], dtype=np.float32)
    w_out = np.asarray(inputs["w_out"], dtype=np.float32)
    nb, tq, _ = query.shape

    in_maps, tkv_c = make_in_maps(query, key_value, kv_mask, w_q, w_k, w_v, w_out, tq)
    nc = _get_nc(tq, tkv_c)
    res = run_bass_kernel_spmd(
        nc, in_maps, list(range(2 * nb)), trace=trace, trace_cores=[0]
    )
    outs = [np.asarray(r["out"]) for r in res.results]
    full = np.stack([outs[2 * b] + outs[2 * b + 1] for b in range(nb)])

    query_mask = np.asarray(inputs["query_mask"])
    if not query_mask.all():
        # masked query rows: reference yields uniform attention over all kv
        for b in range(nb):
            rows = ~query_mask[b]
            if rows.any():
                V = key_value[b] @ w_v  # [tkv, 256]
                meanV = V.mean(axis=0)  # [256]
                group = N_HEADS // NUM_KV_HEADS
                feat = np.concatenate([meanV.reshape(NUM_KV_HEADS, D_K)[h // group]
                                       for h in range(N_HEADS)])
                full[b, rows, :] = feat @ w_out
    return full.astype(np.float32), res


def kernel(**inputs):
    out, _ = _run(inputs, trace=False)
    return out


def kernel_traced(**inputs):
    out, res = _run(inputs, trace=True)
    return out, res


if __name__ == "__main__":
    print("kernel.py is a library; use test.py")


# revision 61
# speedup vs baseline: 1.0069x; 1.0069x over previous
"""Cross-attention (GQA + RoPE) Trainium2 Bass kernel.

Sharding: 8 cores = 4 batches x 2 head-groups.
  core i -> batch b = i // 2, head-group g = i % 2
  Each core computes 8 query heads / 2 kv heads of one batch and a
  row-parallel partial of the output projection; the host sums the two
  partials per batch.

Key optimizations over the v1 baseline:
  * kv compaction: ~50% of kv positions are masked out; the host gathers
    valid positions (and their RoPE phase tables) and pads to a multiple
    of 128 (TKV_C).  Scores / exp / PV / KV-projection all shrink.
  * All inputs are host-pre-arranged into the exact SBUF layout
    [128, c, X] so every DMA moves large contiguous per-partition rows;
    two DMA queues run in parallel (kv-side on gpsimd, q-side on vector).
  * bf16 inputs and intermediates (PSUM stays f32).
  * reciprocal_approx_fast on a [2, t2] packed denominator pair, one
    K=2 broadcast matmul per head-pair (inv0 -> psum rows 0-63,
    inv1 -> rows 64-127).
  * Software pipelining: the next block's Q-projection pairs and the
    previous block's output-projection slices are interleaved between
    attention head-pairs, so the PE never sits in a dedicated
    projection phase; output-projection PSUM->SBUF copies run on the
    Scalar engine.
"""

import math
from contextlib import ExitStack

import numpy as np
import ml_dtypes

import concourse.bass as bass
import concourse.bacc as bacc
import concourse.mybir as mybir
import concourse.tile as tile
from concourse.bass_utils import run_bass_kernel_spmd

F32 = mybir.dt.float32
R32 = mybir.dt.float32r
BF16 = mybir.dt.bfloat16

D_MODEL = 1024
N_HEADS = 16
NUM_KV_HEADS = 4
D_K = 64
ROPE_BASE = 10000.0
TQ = 2048
N_CORES = 8

NEG_BIAS = -30000.0


def _ktiles(tkv_c):
    """K-projection column tiles: two 256-wide leading tiles (so the first
    projection starts as early as possible behind the kv DMA), 512 after."""
    ks = []
    c0 = 0
    while c0 < tkv_c:
        w = min(256 if c0 < 512 else 512, tkv_c - c0)
        ks.append((c0, w))
        c0 += w
    return ks


def build_bass(tq=TQ, tkv_c=1152, t2=1024):
    """Build the single-core SPMD program (same program on all 8 cores)."""
    nc = bacc.Bacc("TRN2", target_bir_lowering=False, debug=False)
    P = 128
    NCH = tkv_c // 128        # attention kv chunks
    NT2 = tq // t2            # tq blocks
    NHALF = t2 // 512         # 512-wide matmul slices per tq block
    NPAIR = 4                 # head-pair tiles per core
    NSLICE = t2 // 128        # output rows per block
    ktiles = _ktiles(tkv_c)

    NQIN = tq // 1024  # q DMA granularity stays 1024 columns
    q_in = [
        nc.dram_tensor(f"q{i}", [P, 8, 1024], BF16, kind="ExternalInput").ap()
        for i in range(NQIN)
    ]
    kv_in = [
        nc.dram_tensor(f"kv_t{i}", [P, 8, w], BF16, kind="ExternalInput").ap()
        for i, (_c0, w) in enumerate(ktiles)
    ]
    wq = nc.dram_tensor("wq", [P, 8, 512], BF16, kind="ExternalInput").ap()
    wk = nc.dram_tensor("wk", [P, 8, 128], BF16, kind="ExternalInput").ap()
    wv = nc.dram_tensor("wv", [P, 8, 128], BF16, kind="ExternalInput").ap()
    wout = nc.dram_tensor("wout", [P, 4, D_MODEL], BF16, kind="ExternalInput").ap()
    cosK = nc.dram_tensor("cosK", [P, tkv_c], BF16, kind="ExternalInput").ap()
    sinK = nc.dram_tensor("sinK", [P, tkv_c], BF16, kind="ExternalInput").ap()
    cosQ = nc.dram_tensor("cosQ", [P, tq], BF16, kind="ExternalInput").ap()
    sinQ = nc.dram_tensor("sinQ", [P, tq], BF16, kind="ExternalInput").ap()
    maskb = nc.dram_tensor("maskb", [P, NCH], F32, kind="ExternalInput").ap()
    e2 = nc.dram_tensor("e2", [64, P], R32, kind="ExternalInput").ap()
    out = nc.dram_tensor("out", [tq, D_MODEL], F32, kind="ExternalOutput").ap()

    with tile.TileContext(nc) as tc, ExitStack() as ctx:
        const = ctx.enter_context(tc.tile_pool(name="const", bufs=1))
        qpool = ctx.enter_context(tc.tile_pool(name="qpool", bufs=1))
        apool = ctx.enter_context(tc.tile_pool(name="apool", bufs=1))
        workp = ctx.enter_context(tc.tile_pool(name="workp", bufs=3))
        ropep = ctx.enter_context(tc.tile_pool(name="ropep", bufs=2))
        big_bufs = 5 if t2 <= 512 else 2
        acc_bufs = 3 if t2 <= 512 else 2
        pp_big = ctx.enter_context(
            tc.tile_pool(name="pp_big", bufs=big_bufs, space="PSUM")
        )
        pp_acc = ctx.enter_context(
            tc.tile_pool(name="pp_acc", bufs=acc_bufs, space="PSUM")
        )

        def MM(out_ap, lhsT, rhs, start, stop, chain=None):
            inst = nc.tensor.matmul(out_ap, lhsT, rhs, start=start, stop=stop)
            if chain is not None:
                tc.chain_iter_dep(chain, inst.ins)
            return inst

        def chain_dve(inst):
            tc.chain_iter_dep("dve_norm", inst.ins)
            return inst

        # ---- constants set up on-engine (no DMA) --------------------------
        Vt = [const.tile([P, NCH * 65], BF16, name=f"Vt{i}") for i in range(2)]
        for i in range(2):
            nc.gpsimd.memset(
                Vt[i].rearrange("p (c k) -> p c k", k=65)[:, :, 64], 1.0
            )

        # ---- kv-side inputs on the gpsimd queue (critical first) ----------
        wk_sb = const.tile([P, 8, 128], BF16)
        nc.gpsimd.dma_start(out=wk_sb, in_=wk)
        wv_sb = const.tile([P, 8, 128], BF16)
        nc.gpsimd.dma_start(out=wv_sb, in_=wv)
        kv_sb = [
            const.tile([P, 8, w], BF16, name=f"kvt{i}")
            for i, (_c0, w) in enumerate(ktiles)
        ]
        nc.gpsimd.dma_start(out=kv_sb[0], in_=kv_in[0])
        cosK_sb = const.tile([P, tkv_c], BF16)
        nc.gpsimd.dma_start(out=cosK_sb, in_=cosK)
        sinK_sb = const.tile([P, tkv_c], BF16)
        nc.gpsimd.dma_start(out=sinK_sb, in_=sinK)
        mask_sb = const.tile([P, NCH], F32)
        nc.gpsimd.dma_start(out=mask_sb, in_=maskb)
        for i in range(1, len(ktiles)):
            nc.gpsimd.dma_start(out=kv_sb[i], in_=kv_in[i])

        # ---- q-side inputs on the scalar queue ----------------------------
        e2_sb = const.tile([64, P], R32)
        nc.scalar.dma_start(out=e2_sb, in_=e2)
        # inv broadcast staging: head0 inv in row 0, head1 inv in row 32,
        # all other rows memset to a safe finite value (multiplied by e2=0).
        invp_tiles = [const.tile([64, t2], R32, name=f"invp{i}") for i in range(4)]
        for tl in invp_tiles:
            nc.gpsimd.memset(tl.bitcast(F32), 1.0)
        wq_sb = const.tile([P, 8, 512], BF16)
        nc.scalar.dma_start(out=wq_sb, in_=wq)
        q_sb = [const.tile([P, 8, 1024], BF16, name=f"qsb{i}") for i in range(NQIN)]
        nc.scalar.dma_start(out=q_sb[0], in_=q_in[0])
        cosQ_sb = const.tile([P, tq], BF16)
        nc.scalar.dma_start(out=cosQ_sb, in_=cosQ)
        sinQ_sb = const.tile([P, tq], BF16)
        nc.scalar.dma_start(out=sinQ_sb, in_=sinQ)
        # late-needed bulk goes on the gpsimd queue tail to balance the two
        wout_sb = const.tile([P, 4, D_MODEL], BF16)
        nc.gpsimd.dma_start(out=wout_sb, in_=wout)
        for i in range(1, NQIN):
            nc.gpsimd.dma_start(out=q_sb[i], in_=q_in[i])

        Kt = const.tile([P, tkv_c], BF16)

        def rope_apply(dest, ps, col0, width, cos_sb, sin_sb):
            """dest[128, width] (SBUF) = rope(ps[128, width] PSUM), positions
            col0..col0+width. Rows are two stacked heads, each [x1(32); x2(32)]."""
            cs = cos_sb[:, col0 : col0 + width]
            t_cos = ropep.tile([P, t2], F32, tag="rope", name="t_cos")
            t_u = ropep.tile([P, t2], F32, tag="rope", name="t_u")
            tc_ = t_cos[:, :width]
            tu_ = t_u[:, :width]
            nc.vector.tensor_mul(tc_, ps, cs)
            for b0 in (0, 64):
                # sin rows [b0:b0+32] = -sin, [b0+32:b0+64] = +sin
                nc.vector.tensor_mul(
                    tu_[b0 : b0 + 32, :],
                    ps[b0 + 32 : b0 + 64, :],
                    sin_sb[b0 : b0 + 32, col0 : col0 + width],
                )
                nc.vector.tensor_mul(
                    tu_[b0 + 32 : b0 + 64, :],
                    ps[b0 : b0 + 32, :],
                    sin_sb[b0 + 32 : b0 + 64, col0 : col0 + width],
                )
            with nc.allow_low_precision("rope output bf16"):
                nc.vector.tensor_add(dest, tc_, tu_)

        # ---- phase KV: K/V projections (invoked from the pipeline) --------
        def kv_tile(kt):
            kc0, kw = ktiles[kt]
            cols = slice(kc0, kc0 + kw)
            ps_k = pp_big.tile([P, 512], F32, tag="big", name="ps_k")
            pk = ps_k[:, :kw]
            for d in range(8):
                MM(pk, wk_sb[:, d, :], kv_sb[kt][:, d, :], d == 0, d == 7)
            rope_apply(Kt[:, cols], pk, kc0, kw, cosK_sb, sinK_sb)
            for s in range(kw // 128):
                ps_v = pp_big.tile([P, 512], F32, tag="big", name="ps_v")
                pv = ps_v[:, 0:128]
                lv = slice(s * 128, (s + 1) * 128)
                for d in range(8):
                    MM(pv, kv_sb[kt][:, d, lv], wv_sb[:, d, :], d == 0, d == 7)
                c = kc0 // 128 + s
                with nc.allow_low_precision("V bf16"):
                    nc.vector.tensor_copy(
                        out=Vt[0][:, c * 65 : c * 65 + 64], in_=pv[:, 0:64]
                    )
                    nc.vector.tensor_copy(
                        out=Vt[1][:, c * 65 : c * 65 + 64], in_=pv[:, 64:128]
                    )

        # ---- double-generation Qt / attnT tiles ---------------------------
        Qt = [
            [
                qpool.tile([P, t2], BF16, tag=f"Q{j}g{ggen}", name=f"Qt{j}g{ggen}")
                for j in range(NPAIR)
            ]
            for ggen in range(2)
        ]
        At = [
            [
                apool.tile([P, t2], BF16, tag=f"A{j}g{ggen}", name=f"At{j}g{ggen}")
                for j in range(NPAIR)
            ]
            for ggen in range(2)
        ]
        pending = []
        pair_seq = [0]

        def qproj_pair(it2, j):
            ps_q = pp_big.tile([P, t2], F32, tag="big", name="ps_q")
            for half in range(NHALF):
                c0 = it2 * t2 + half * 512
                for d in range(8):
                    MM(
                        ps_q[:, half * 512 : (half + 1) * 512],
                        wq_sb[:, d, j * 128 : (j + 1) * 128],
                        q_sb[c0 // 1024][:, d, c0 % 1024 : c0 % 1024 + 512],
                        d == 0,
                        d == 7,
                    )
            rope_apply(Qt[it2 % 2][j], ps_q, it2 * t2, t2, cosQ_sb, sinQ_sb)

        def flush_norm():
            if not pending:
                return
            U0, U1, invp, j_, attn_cur = pending.pop(0)
            Us = (U0, U1)
            for half in range(NHALF):
                hs = slice(half * 512, (half + 1) * 512)
                ps_b = pp_big.tile([P, 512], F32, tag="big", name="ps_b")
                MM(ps_b, e2_sb, invp[:, hs], True, True, chain="pe_attn")
                for ab, base in ((0, 0), (1, 64)):
                    with nc.allow_low_precision("attnT bf16"):
                        chain_dve(
                            nc.vector.tensor_mul(
                                attn_cur[j_][base : base + 64, hs],
                                Us[ab][0:64, hs],
                                ps_b[base : base + 64, :],
                            )
                        )

        def outproj_slices(it2, slices):
            attn_cur = At[it2 % 2]
            for s in slices:
                ob = ropep.tile([P, D_MODEL], F32, tag="ob", name="ob", bufs=2)
                for n in range(2):
                    ps_f = pp_big.tile([P, 512], F32, tag="big", name="ps_f")
                    for p_ in range(NPAIR):
                        MM(
                            ps_f,
                            attn_cur[p_][:, s * 128 : (s + 1) * 128],
                            wout_sb[:, p_, n * 512 : (n + 1) * 512],
                            p_ == 0,
                            p_ == NPAIR - 1,
                        )
                    nc.scalar.copy(out=ob[:, n * 512 : (n + 1) * 512], in_=ps_f)
                r0 = it2 * t2 + s * 128
                eng = nc.sync if s % 2 == 0 else nc.gpsimd
                eng.dma_start(out=out[r0 : r0 + 128, :], in_=ob)

        def make_pair(it2, j):
            """Resumable attention for head-pair j of block it2: run(c_lo,
            c_hi) emits chunk work; finish() emits the PV tail and the
            normalization prologue (U/den copies, reciprocal, inv pack)."""
            Qt_cur = Qt[it2 % 2]
            attn_cur = At[it2 % 2]
            heads = [(j, 0, 0), (j + 4, 1, 64)]  # (head, kvh, base)
            ps_os = [
                pp_acc.tile([65, t2], F32, tag="acc", name=f"ps_o{ab}")
                for ab in range(2)
            ]
            st = {"prev": None}

            def emit_pv(c_, exs_):
                for ab, (_h, kvh, _base) in enumerate(heads):
                    for half in range(NHALF):
                        MM(
                            ps_os[ab][:, half * 512 : (half + 1) * 512],
                            Vt[kvh][:, c_ * 65 : c_ * 65 + 65],
                            exs_[ab][:, half * 512 : (half + 1) * 512],
                            c_ == 0,
                            c_ == NCH - 1,
                            chain="pe_attn",
                        )

            def run(c_lo, c_hi):
                # PV lags the scores by one chunk so no PE instruction ever
                # reaches the queue head with an unresolved wait.
                for c in range(c_lo, c_hi):
                    exs = []
                    for ab, (_h, kvh, base) in enumerate(heads):
                        ps_s = pp_big.tile([P, t2], F32, tag="big", name="ps_s")
                        for half in range(NHALF):
                            MM(
                                ps_s[:, half * 512 : (half + 1) * 512],
                                Kt[base : base + 64, c * 128 : (c + 1) * 128],
                                Qt_cur[j][base : base + 64, half * 512 : (half + 1) * 512],
                                True,
                                True,
                                chain="pe_attn",
                            )
                        ex = workp.tile([P, t2], BF16, tag="expT", name="ex", bufs=8)
                        nc.scalar.activation(
                            out=ex,
                            in_=ps_s,
                            func=mybir.ActivationFunctionType.Exp,
                            bias=mask_sb[:, c : c + 1],
                            scale=0.125,
                        )
                        exs.append(ex)
                    if st["prev"] is not None:
                        emit_pv(c - 1, st["prev"])
                    st["prev"] = exs

            def finish():
                emit_pv(NCH - 1, st["prev"])
                # flush the previous pair first: its bcast matmul runs now
                # (reciprocal long done), and its muls free ps_b slots early.
                while pending:
                    flush_norm()
                # accumulator copies BEFORE the reciprocal: the in-order DVE
                # must release both PSUM slots promptly.
                invp = invp_tiles[pair_seq[0] % 4]
                pair_seq[0] += 1
                Us = []
                for ab in range(2):
                    U = workp.tile([64, t2], F32, tag="unorm", name="U", bufs=6)
                    chain_dve(nc.vector.tensor_copy(out=U, in_=ps_os[ab][0:64, :]))
                    den = workp.tile([1, t2], F32, tag="den", name="den", bufs=4)
                    chain_dve(
                        nc.vector.tensor_copy(out=den, in_=ps_os[ab][64:65, :])
                    )
                    inv_f = workp.tile([1, t2], F32, tag="invf", name="inv_f", bufs=4)
                    chain_dve(nc.vector.reciprocal_approx_fast(out=inv_f, in_=den))
                    with nc.allow_low_precision("f32r softmax denom"):
                        chain_dve(
                            nc.vector.tensor_copy(
                                out=invp[32 * ab : 32 * ab + 1, :], in_=inv_f
                            )
                        )
                    Us.append(U)
                pending.append((Us[0], Us[1], invp, j, attn_cur))

            return run, finish

        # ---- pipeline -----------------------------------------------------
        qtasks = [(b2, jj) for b2 in range(NT2) for jj in range(NPAIR)]

        def pop_q():
            if qtasks:
                qproj_pair(*qtasks.pop(0))

        # Block-0 pair-0 attention streams directly behind the KV tiles so
        # scoring starts as soon as the first kv tile + Q pair are ready.
        ranges = []
        acc0 = 0
        for _kc0, kw in ktiles:
            ranges.append((acc0, acc0 + kw // 128))
            acc0 += kw // 128
        for kt in range(len(ktiles)):
            kv_tile(kt)
            if kt < 2:
                pop_q()
        # one Q-projection task and (for it2 > 0) the previous block's
        # output slices per attention pair end.
        for it2 in range(NT2):
            nsl = 0
            for j in range(NPAIR):
                run, fin = make_pair(it2, j)
                run(0, NCH)
                fin()
                pop_q()
                if it2 > 0:
                    take = NSLICE * (j + 1) // NPAIR
                    outproj_slices(it2 - 1, range(nsl, take))
                    nsl = take
        while pending:
            flush_norm()
        outproj_slices(NT2 - 1, range(NSLICE))

    nc.compile()
    return nc


# ---------------------------------------------------------------------------
# host-side sharding / prep
# ---------------------------------------------------------------------------

_HEAD_PERM = [0, 4, 1, 5, 2, 6, 3, 7]  # local head order inside pair tiles


def _rope_tables(positions):
    """cos/sin tables [128, len(positions)] with the sign pattern baked in."""
    theta = ROPE_BASE ** (-np.arange(0, D_K, 2, dtype=np.float64) / D_K)  # [32]
    ang = positions.astype(np.float64)[:, None] * theta[None, :]  # [T,32]
    c = np.cos(ang).T.astype(np.float32)  # [32, T]
    s = np.sin(ang).T.astype(np.float32)
    cosF = np.concatenate([c, c, c, c], axis=0)
    sinF = np.concatenate([-s, s, -s, s], axis=0)
    return np.ascontiguousarray(cosF), np.ascontiguousarray(sinF)


def _pack8(a, c, width):
    """[c*128, width] -> [128, c, width] with row d = c_idx*128 + p."""
    return np.ascontiguousarray(a.reshape(c, 128, width).transpose(1, 0, 2))


def make_in_maps(query, key_value, kv_mask, w_q, w_k, w_v, w_out, tq=TQ):
    nb = query.shape[0]
    bf = ml_dtypes.bfloat16

    idxs = [np.nonzero(kv_mask[b])[0] for b in range(nb)]
    nmax = max((len(i) for i in idxs), default=1)
    tkv_c = max(256, int(math.ceil(max(nmax, 1) / 128.0)) * 128)
    nch = tkv_c // 128

    cosQ, sinQ = _rope_tables(np.arange(tq))
    cosQ_bf = cosQ.astype(bf)
    sinQ_bf = sinQ.astype(bf)
    e2 = np.zeros((64, 128), np.float32)
    e2[0, 0:64] = 1.0
    e2[32, 64:128] = 1.0

    col_perm = np.concatenate(
        [np.arange(h * D_K, (h + 1) * D_K) for h in _HEAD_PERM]
    )
    in_maps = []
    for core in range(2 * nb):
        b = core // 2
        g = core % 2
        idx = idxs[b]
        nv = len(idx)

        kv_c = np.zeros((tkv_c, D_MODEL), np.float32)
        kv_c[:nv] = key_value[b][idx]
        kvT = np.ascontiguousarray(kv_c.T)  # [1024, tkv_c]

        pos = np.zeros(tkv_c, np.int64)
        pos[:nv] = idx
        cosK, sinK = _rope_tables(pos)

        maskb = np.full(tkv_c, NEG_BIAS, np.float32)
        maskb[:nv] = 0.0
        maskb = np.ascontiguousarray(maskb.reshape(nch, 128).T)

        qT = np.ascontiguousarray(query[b].T)  # [1024, tq]

        wq_g = w_q[:, g * 512 : (g + 1) * 512][:, col_perm]
        wk_g = w_k[:, g * 128 : (g + 1) * 128]
        wv_g = w_v[:, g * 128 : (g + 1) * 128]
        wout_g = w_out[g * 512 : (g + 1) * 512, :][col_perm, :]

        m = {
            "wq": _pack8(np.ascontiguousarray(wq_g), 8, 512).astype(bf),
            "wk": _pack8(np.ascontiguousarray(wk_g), 8, 128).astype(bf),
            "wv": _pack8(np.ascontiguousarray(wv_g), 8, 128).astype(bf),
            "wout": _pack8(np.ascontiguousarray(wout_g), 4, D_MODEL).astype(bf),
            "cosK": cosK.astype(bf),
            "sinK": sinK.astype(bf),
            "cosQ": cosQ_bf,
            "sinQ": sinQ_bf,
            "maskb": maskb,
            "e2": e2,
        }
        for i, (c0, w) in enumerate(_ktiles(tkv_c)):
            m[f"kv_t{i}"] = _pack8(
                np.ascontiguousarray(kvT[:, c0 : c0 + w]), 8, w
            ).astype(bf)
        for i in range(tq // 1024):
            m[f"q{i}"] = _pack8(
                np.ascontiguousarray(qT[:, i * 1024 : (i + 1) * 1024]), 8, 1024
            ).astype(bf)
        in_maps.append(m)
    return in_maps, tkv_c


_NC_CACHE = {}


T2 = 512


def _get_nc(tq, tkv_c):
    key = (tq, tkv_c, T2)
    if key not in _NC_CACHE:
        _NC_CACHE[key] = build_bass(tq, tkv_c, T2)
    return _NC_CACHE[key]


def _run(inputs, trace=False):
    query = np.asarray(inputs["query"], dtype=np.float32)
    key_value = np.asarray(inputs["key_value"], dtype=np.float32)
    kv_mask = np.asarray(inputs["kv_mask"])
    w_q = np.asarray(inputs["w_q"], dtype=np.float32)
    w_k = np.asarray(inputs["w_k"], dtype=np.float32)
    w_v = np.asarray(inputs["w_v"], dtype=np.float32)
    w_out = np.asarray(inputs["w_out"], dtype=np.float32)
    nb, tq, _ = query.shape

    in_maps, tkv_c = make_in_maps(query, key_value, kv_mask, w_q, w_k, w_v, w_out, tq)
    nc = _get_nc(tq, tkv_c)
    res = run_bass_kernel_spmd(
        nc, in_maps, list(range(2 * nb)), trace=trace, trace_cores=[0]
    )
    outs = [np.asarray(r["out"]) for r in res.results]
    full = np.stack([outs[2 * b] + outs[2 * b + 1] for b in range(nb)])

    query_mask = np.asarray(inputs["query_mask"])
    if not query_mask.all():
        # masked query rows: reference yields uniform attention over all kv
        for b in range(nb):
            rows = ~query_mask[b]
            if rows.any():
                V = key_value[b] @ w_v  # [tkv, 256]
                meanV = V.mean(axis=0)  # [256]
                group = N_HEADS // NUM_KV_HEADS
                feat = np.concatenate([meanV.reshape(NUM_KV_HEADS, D_K)[h // group]
                                       for h in range(N_HEADS)])
                full[b, rows, :] = feat @ w_out
    return full.astype(np.float32), res


def kernel(**inputs):
    out, _ = _run(inputs, trace=False)
    return out


def kernel_traced(**inputs):
    out, res = _run(inputs, trace=True)
    return out, res


if __name__ == "__main__":
    print("kernel.py is a library; use test.py")


# revision 62
# speedup vs baseline: 1.0128x; 1.0059x over previous
"""Cross-attention (GQA + RoPE) Trainium2 Bass kernel.

Sharding: 8 cores = 4 batches x 2 head-groups.
  core i -> batch b = i // 2, head-group g = i % 2
  Each core computes 8 query heads / 2 kv heads of one batch and a
  row-parallel partial of the output projection; the host sums the two
  partials per batch.

Key optimizations over the v1 baseline:
  * kv compaction: ~50% of kv positions are masked out; the host gathers
    valid positions (and their RoPE phase tables) and pads to a multiple
    of 128 (TKV_C).  Scores / exp / PV / KV-projection all shrink.
  * All inputs are host-pre-arranged into the exact SBUF layout
    [128, c, X] so every DMA moves large contiguous per-partition rows;
    two DMA queues run in parallel (kv-side on gpsimd, q-side on vector).
  * bf16 inputs and intermediates (PSUM stays f32).
  * reciprocal_approx_fast on a [2, t2] packed denominator pair, one
    K=2 broadcast matmul per head-pair (inv0 -> psum rows 0-63,
    inv1 -> rows 64-127).
  * Software pipelining: the next block's Q-projection pairs and the
    previous block's output-projection slices are interleaved between
    attention head-pairs, so the PE never sits in a dedicated
    projection phase; output-projection PSUM->SBUF copies run on the
    Scalar engine.
"""

import math
from contextlib import ExitStack

import numpy as np
import ml_dtypes

import concourse.bass as bass
import concourse.bacc as bacc
import concourse.mybir as mybir
import concourse.tile as tile
from concourse.bass_utils import run_bass_kernel_spmd

F32 = mybir.dt.float32
R32 = mybir.dt.float32r
BF16 = mybir.dt.bfloat16

D_MODEL = 1024
N_HEADS = 16
NUM_KV_HEADS = 4
D_K = 64
ROPE_BASE = 10000.0
TQ = 2048
N_CORES = 8

NEG_BIAS = -30000.0


def _ktiles(tkv_c):
    """K-projection column tiles: two 256-wide leading tiles (so the first
    projection starts as early as possible behind the kv DMA), 512 after."""
    ks = []
    c0 = 0
    while c0 < tkv_c:
        w = min(256 if c0 < 512 else 512, tkv_c - c0)
        ks.append((c0, w))
        c0 += w
    return ks


def build_bass(tq=TQ, tkv_c=1152, t2=1024):
    """Build the single-core SPMD program (same program on all 8 cores)."""
    nc = bacc.Bacc("TRN2", target_bir_lowering=False, debug=False)
    P = 128
    NCH = tkv_c // 128        # attention kv chunks
    NT2 = tq // t2            # tq blocks
    NHALF = t2 // 512         # 512-wide matmul slices per tq block
    NPAIR = 4                 # head-pair tiles per core
    NSLICE = t2 // 128        # output rows per block
    ktiles = _ktiles(tkv_c)

    NQIN = tq // 1024  # q DMA granularity stays 1024 columns
    q_in = [
        nc.dram_tensor(f"q{i}", [P, 8, 1024], BF16, kind="ExternalInput").ap()
        for i in range(NQIN)
    ]
    kv_in = [
        nc.dram_tensor(f"kv_t{i}", [P, 8, w], BF16, kind="ExternalInput").ap()
        for i, (_c0, w) in enumerate(ktiles)
    ]
    wq = nc.dram_tensor("wq", [P, 8, 512], BF16, kind="ExternalInput").ap()
    wk = nc.dram_tensor("wk", [P, 8, 128], BF16, kind="ExternalInput").ap()
    wv = nc.dram_tensor("wv", [P, 8, 128], BF16, kind="ExternalInput").ap()
    wout = nc.dram_tensor("wout", [P, 4, D_MODEL], BF16, kind="ExternalInput").ap()
    cosK = nc.dram_tensor("cosK", [P, tkv_c], BF16, kind="ExternalInput").ap()
    sinK = nc.dram_tensor("sinK", [P, tkv_c], BF16, kind="ExternalInput").ap()
    cosQ = nc.dram_tensor("cosQ", [P, tq], BF16, kind="ExternalInput").ap()
    sinQ = nc.dram_tensor("sinQ", [P, tq], BF16, kind="ExternalInput").ap()
    maskb = nc.dram_tensor("maskb", [P, NCH], F32, kind="ExternalInput").ap()
    e2 = nc.dram_tensor("e2", [64, P], R32, kind="ExternalInput").ap()
    out = nc.dram_tensor("out", [tq, D_MODEL], F32, kind="ExternalOutput").ap()

    with tile.TileContext(nc) as tc, ExitStack() as ctx:
        const = ctx.enter_context(tc.tile_pool(name="const", bufs=1))
        qpool = ctx.enter_context(tc.tile_pool(name="qpool", bufs=1))
        apool = ctx.enter_context(tc.tile_pool(name="apool", bufs=1))
        workp = ctx.enter_context(tc.tile_pool(name="workp", bufs=3))
        ropep = ctx.enter_context(tc.tile_pool(name="ropep", bufs=2))
        big_bufs = 5 if t2 <= 512 else 2
        acc_bufs = 3 if t2 <= 512 else 2
        pp_big = ctx.enter_context(
            tc.tile_pool(name="pp_big", bufs=big_bufs, space="PSUM")
        )
        pp_acc = ctx.enter_context(
            tc.tile_pool(name="pp_acc", bufs=acc_bufs, space="PSUM")
        )

        def MM(out_ap, lhsT, rhs, start, stop, chain=None):
            inst = nc.tensor.matmul(out_ap, lhsT, rhs, start=start, stop=stop)
            if chain is not None:
                tc.chain_iter_dep(chain, inst.ins)
            return inst

        def chain_dve(inst):
            tc.chain_iter_dep("dve_norm", inst.ins)
            return inst

        # ---- constants set up on-engine (no DMA) --------------------------
        Vt = [const.tile([P, NCH * 65], BF16, name=f"Vt{i}") for i in range(2)]
        for i in range(2):
            nc.gpsimd.memset(
                Vt[i].rearrange("p (c k) -> p c k", k=65)[:, :, 64], 1.0
            )

        # ---- kv-side inputs on the gpsimd queue (critical first) ----------
        wk_sb = const.tile([P, 8, 128], BF16)
        nc.gpsimd.dma_start(out=wk_sb, in_=wk)
        wv_sb = const.tile([P, 8, 128], BF16)
        nc.gpsimd.dma_start(out=wv_sb, in_=wv)
        kv_sb = [
            const.tile([P, 8, w], BF16, name=f"kvt{i}")
            for i, (_c0, w) in enumerate(ktiles)
        ]
        nc.gpsimd.dma_start(out=kv_sb[0], in_=kv_in[0])
        cosK_sb = const.tile([P, tkv_c], BF16)
        nc.gpsimd.dma_start(out=cosK_sb, in_=cosK)
        sinK_sb = const.tile([P, tkv_c], BF16)
        nc.gpsimd.dma_start(out=sinK_sb, in_=sinK)
        mask_sb = const.tile([P, NCH], F32)
        nc.gpsimd.dma_start(out=mask_sb, in_=maskb)
        for i in range(1, len(ktiles)):
            nc.gpsimd.dma_start(out=kv_sb[i], in_=kv_in[i])

        # ---- q-side inputs on the scalar queue ----------------------------
        e2_sb = const.tile([64, P], R32)
        nc.scalar.dma_start(out=e2_sb, in_=e2)
        # inv broadcast staging: head0 inv in row 0, head1 inv in row 32,
        # all other rows memset to a safe finite value (multiplied by e2=0).
        invp_tiles = [const.tile([64, t2], R32, name=f"invp{i}") for i in range(4)]
        for tl in invp_tiles:
            nc.gpsimd.memset(tl.bitcast(F32), 1.0)
        wq_sb = const.tile([P, 8, 512], BF16)
        nc.scalar.dma_start(out=wq_sb, in_=wq)
        q_sb = [const.tile([P, 8, 1024], BF16, name=f"qsb{i}") for i in range(NQIN)]
        nc.scalar.dma_start(out=q_sb[0], in_=q_in[0])
        cosQ_sb = const.tile([P, tq], BF16)
        nc.scalar.dma_start(out=cosQ_sb, in_=cosQ)
        sinQ_sb = const.tile([P, tq], BF16)
        nc.scalar.dma_start(out=sinQ_sb, in_=sinQ)
        # late-needed bulk goes on the gpsimd queue tail to balance the two
        wout_sb = const.tile([P, 4, D_MODEL], BF16)
        nc.gpsimd.dma_start(out=wout_sb, in_=wout)
        for i in range(1, NQIN):
            nc.gpsimd.dma_start(out=q_sb[i], in_=q_in[i])

        Kt = const.tile([P, tkv_c], BF16)

        def rope_apply(dest, ps, col0, width, cos_sb, sin_sb):
            """dest[128, width] (SBUF) = rope(ps[128, width] PSUM), positions
            col0..col0+width. Rows are two stacked heads, each [x1(32); x2(32)]."""
            cs = cos_sb[:, col0 : col0 + width]
            t_cos = ropep.tile([P, t2], F32, tag="rope", name="t_cos")
            t_u = ropep.tile([P, t2], F32, tag="rope", name="t_u")
            tc_ = t_cos[:, :width]
            tu_ = t_u[:, :width]
            nc.vector.tensor_mul(tc_, ps, cs)
            for b0 in (0, 64):
                # sin rows [b0:b0+32] = -sin, [b0+32:b0+64] = +sin
                nc.vector.tensor_mul(
                    tu_[b0 : b0 + 32, :],
                    ps[b0 + 32 : b0 + 64, :],
                    sin_sb[b0 : b0 + 32, col0 : col0 + width],
                )
                nc.vector.tensor_mul(
                    tu_[b0 + 32 : b0 + 64, :],
                    ps[b0 : b0 + 32, :],
                    sin_sb[b0 + 32 : b0 + 64, col0 : col0 + width],
                )
            with nc.allow_low_precision("rope output bf16"):
                nc.vector.tensor_add(dest, tc_, tu_)

        # ---- phase KV: K/V projections (invoked from the pipeline) --------
        def kv_tile(kt):
            kc0, kw = ktiles[kt]
            cols = slice(kc0, kc0 + kw)
            ps_k = pp_big.tile([P, 512], F32, tag="big", name="ps_k")
            pk = ps_k[:, :kw]
            for d in range(8):
                MM(pk, wk_sb[:, d, :], kv_sb[kt][:, d, :], d == 0, d == 7)
            rope_apply(Kt[:, cols], pk, kc0, kw, cosK_sb, sinK_sb)
            for s in range(kw // 128):
                ps_v = pp_big.tile([P, 512], F32, tag="big", name="ps_v")
                pv = ps_v[:, 0:128]
                lv = slice(s * 128, (s + 1) * 128)
                for d in range(8):
                    MM(pv, kv_sb[kt][:, d, lv], wv_sb[:, d, :], d == 0, d == 7)
                c = kc0 // 128 + s
                with nc.allow_low_precision("V bf16"):
                    nc.vector.tensor_copy(
                        out=Vt[0][:, c * 65 : c * 65 + 64], in_=pv[:, 0:64]
                    )
                    nc.vector.tensor_copy(
                        out=Vt[1][:, c * 65 : c * 65 + 64], in_=pv[:, 64:128]
                    )

        # ---- double-generation Qt / attnT tiles ---------------------------
        Qt = [
            [
                qpool.tile([P, t2], BF16, tag=f"Q{j}g{ggen}", name=f"Qt{j}g{ggen}")
                for j in range(NPAIR)
            ]
            for ggen in range(2)
        ]
        At = [
            [
                apool.tile([P, t2], BF16, tag=f"A{j}g{ggen}", name=f"At{j}g{ggen}")
                for j in range(NPAIR)
            ]
            for ggen in range(2)
        ]
        pending = []
        pair_seq = [0]

        def qproj_pair(it2, j):
            ps_q = pp_big.tile([P, t2], F32, tag="big", name="ps_q")
            for half in range(NHALF):
                c0 = it2 * t2 + half * 512
                for d in range(8):
                    MM(
                        ps_q[:, half * 512 : (half + 1) * 512],
                        wq_sb[:, d, j * 128 : (j + 1) * 128],
                        q_sb[c0 // 1024][:, d, c0 % 1024 : c0 % 1024 + 512],
                        d == 0,
                        d == 7,
                    )
            rope_apply(Qt[it2 % 2][j], ps_q, it2 * t2, t2, cosQ_sb, sinQ_sb)

        def flush_norm():
            if not pending:
                return
            U0, U1, invp, j_, attn_cur = pending.pop(0)
            Us = (U0, U1)
            for half in range(NHALF):
                hs = slice(half * 512, (half + 1) * 512)
                ps_b = pp_big.tile([P, 512], F32, tag="big", name="ps_b")
                MM(ps_b, e2_sb, invp[:, hs], True, True, chain="pe_attn")
                for ab, base in ((0, 0), (1, 64)):
                    with nc.allow_low_precision("attnT bf16"):
                        chain_dve(
                            nc.vector.tensor_mul(
                                attn_cur[j_][base : base + 64, hs],
                                Us[ab][0:64, hs],
                                ps_b[base : base + 64, :],
                            )
                        )

        def outproj_slices(it2, slices):
            attn_cur = At[it2 % 2]
            for s in slices:
                ob = ropep.tile([P, D_MODEL], F32, tag="ob", name="ob", bufs=2)
                for n in range(2):
                    ps_f = pp_big.tile([P, 512], F32, tag="big", name="ps_f")
                    for p_ in range(NPAIR):
                        MM(
                            ps_f,
                            attn_cur[p_][:, s * 128 : (s + 1) * 128],
                            wout_sb[:, p_, n * 512 : (n + 1) * 512],
                            p_ == 0,
                            p_ == NPAIR - 1,
                        )
                    nc.scalar.copy(out=ob[:, n * 512 : (n + 1) * 512], in_=ps_f)
                r0 = it2 * t2 + s * 128
                eng = nc.sync if s % 2 == 0 else nc.gpsimd
                eng.dma_start(out=out[r0 : r0 + 128, :], in_=ob)

        def make_pair(it2, j):
            """Resumable attention for head-pair j of block it2: run(c_lo,
            c_hi) emits chunk work; finish() emits the PV tail and the
            normalization prologue (U/den copies, reciprocal, inv pack)."""
            Qt_cur = Qt[it2 % 2]
            attn_cur = At[it2 % 2]
            heads = [(j, 0, 0), (j + 4, 1, 64)]  # (head, kvh, base)
            ps_os = [
                pp_acc.tile([65, t2], F32, tag="acc", name=f"ps_o{ab}")
                for ab in range(2)
            ]
            st = {"prev": None}

            def emit_pv(c_, exs_):
                for ab, (_h, kvh, _base) in enumerate(heads):
                    for half in range(NHALF):
                        MM(
                            ps_os[ab][:, half * 512 : (half + 1) * 512],
                            Vt[kvh][:, c_ * 65 : c_ * 65 + 65],
                            exs_[ab][:, half * 512 : (half + 1) * 512],
                            c_ == 0,
                            c_ == NCH - 1,
                            chain="pe_attn",
                        )

            def run(c_lo, c_hi):
                # PV lags the scores by one chunk so no PE instruction ever
                # reaches the queue head with an unresolved wait.
                for c in range(c_lo, c_hi):
                    exs = []
                    for ab, (_h, kvh, base) in enumerate(heads):
                        ps_s = pp_big.tile([P, t2], F32, tag="big", name="ps_s")
                        for half in range(NHALF):
                            MM(
                                ps_s[:, half * 512 : (half + 1) * 512],
                                Kt[base : base + 64, c * 128 : (c + 1) * 128],
                                Qt_cur[j][base : base + 64, half * 512 : (half + 1) * 512],
                                True,
                                True,
                                chain="pe_attn",
                            )
                        ex = workp.tile([P, t2], BF16, tag="expT", name="ex", bufs=8)
                        nc.scalar.activation(
                            out=ex,
                            in_=ps_s,
                            func=mybir.ActivationFunctionType.Exp,
                            bias=mask_sb[:, c : c + 1],
                            scale=0.125,
                        )
                        exs.append(ex)
                    if st["prev"] is not None:
                        emit_pv(c - 1, st["prev"])
                    st["prev"] = exs

            def finish():
                emit_pv(NCH - 1, st["prev"])
                # flush the previous pair first: its bcast matmul runs now
                # (reciprocal long done), and its muls free ps_b slots early.
                while pending:
                    flush_norm()
                # accumulator copies BEFORE the reciprocal: the in-order DVE
                # must release both PSUM slots promptly.
                invp = invp_tiles[pair_seq[0] % 4]
                pair_seq[0] += 1
                Us = []
                for ab in range(2):
                    U = workp.tile([64, t2], F32, tag="unorm", name="U", bufs=4)
                    chain_dve(nc.vector.tensor_copy(out=U, in_=ps_os[ab][0:64, :]))
                    den = workp.tile([1, t2], F32, tag="den", name="den", bufs=4)
                    chain_dve(
                        nc.vector.tensor_copy(out=den, in_=ps_os[ab][64:65, :])
                    )
                    inv_f = workp.tile([1, t2], F32, tag="invf", name="inv_f", bufs=4)
                    chain_dve(nc.vector.reciprocal_approx_fast(out=inv_f, in_=den))
                    with nc.allow_low_precision("f32r softmax denom"):
                        chain_dve(
                            nc.vector.tensor_copy(
                                out=invp[32 * ab : 32 * ab + 1, :], in_=inv_f
                            )
                        )
                    Us.append(U)
                pending.append((Us[0], Us[1], invp, j, attn_cur))

            return run, finish

        # ---- pipeline -----------------------------------------------------
        qtasks = [(b2, jj) for b2 in range(NT2) for jj in range(NPAIR)]

        def pop_q():
            if qtasks:
                qproj_pair(*qtasks.pop(0))

        # Block-0 pair-0 attention streams directly behind the KV tiles so
        # scoring starts as soon as the first kv tile + Q pair are ready.
        ranges = []
        acc0 = 0
        for _kc0, kw in ktiles:
            ranges.append((acc0, acc0 + kw // 128))
            acc0 += kw // 128
        for kt in range(len(ktiles)):
            kv_tile(kt)
            if kt < 2:
                pop_q()
        # one Q-projection task and (for it2 > 0) the previous block's
        # output slices per attention pair end.
        for it2 in range(NT2):
            nsl = 0
            for j in range(NPAIR):
                run, fin = make_pair(it2, j)
                run(0, NCH)
                fin()
                pop_q()
                if it2 > 0:
                    take = NSLICE * (j + 1) // NPAIR
                    outproj_slices(it2 - 1, range(nsl, take))
                    nsl = take
        while pending:
            flush_norm()
        outproj_slices(NT2 - 1, range(NSLICE))

    nc.compile()
    return nc


# ---------------------------------------------------------------------------
# host-side sharding / prep
# ---------------------------------------------------------------------------

_HEAD_PERM = [0, 4, 1, 5, 2, 6, 3, 7]  # local head order inside pair tiles


def _rope_tables(positions):
    """cos/sin tables [128, len(positions)] with the sign pattern baked in."""
    theta = ROPE_BASE ** (-np.arange(0, D_K, 2, dtype=np.float64) / D_K)  # [32]
    ang = positions.astype(np.float64)[:, None] * theta[None, :]  # [T,32]
    c = np.cos(ang).T.astype(np.float32)  # [32, T]
    s = np.sin(ang).T.astype(np.float32)
    cosF = np.concatenate([c, c, c, c], axis=0)
    sinF = np.concatenate([-s, s, -s, s], axis=0)
    return np.ascontiguousarray(cosF), np.ascontiguousarray(sinF)


def _pack8(a, c, width):
    """[c*128, width] -> [128, c, width] with row d = c_idx*128 + p."""
    return np.ascontiguousarray(a.reshape(c, 128, width).transpose(1, 0, 2))


def make_in_maps(query, key_value, kv_mask, w_q, w_k, w_v, w_out, tq=TQ):
    nb = query.shape[0]
    bf = ml_dtypes.bfloat16

    idxs = [np.nonzero(kv_mask[b])[0] for b in range(nb)]
    nmax = max((len(i) for i in idxs), default=1)
    tkv_c = max(256, int(math.ceil(max(nmax, 1) / 128.0)) * 128)
    nch = tkv_c // 128

    cosQ, sinQ = _rope_tables(np.arange(tq))
    cosQ_bf = cosQ.astype(bf)
    sinQ_bf = sinQ.astype(bf)
    e2 = np.zeros((64, 128), np.float32)
    e2[0, 0:64] = 1.0
    e2[32, 64:128] = 1.0

    col_perm = np.concatenate(
        [np.arange(h * D_K, (h + 1) * D_K) for h in _HEAD_PERM]
    )
    in_maps = []
    for core in range(2 * nb):
        b = core // 2
        g = core % 2
        idx = idxs[b]
        nv = len(idx)

        kv_c = np.zeros((tkv_c, D_MODEL), np.float32)
        kv_c[:nv] = key_value[b][idx]
        kvT = np.ascontiguousarray(kv_c.T)  # [1024, tkv_c]

        pos = np.zeros(tkv_c, np.int64)
        pos[:nv] = idx
        cosK, sinK = _rope_tables(pos)

        maskb = np.full(tkv_c, NEG_BIAS, np.float32)
        maskb[:nv] = 0.0
        maskb = np.ascontiguousarray(maskb.reshape(nch, 128).T)

        qT = np.ascontiguousarray(query[b].T)  # [1024, tq]

        wq_g = w_q[:, g * 512 : (g + 1) * 512][:, col_perm]
        wk_g = w_k[:, g * 128 : (g + 1) * 128]
        wv_g = w_v[:, g * 128 : (g + 1) * 128]
        wout_g = w_out[g * 512 : (g + 1) * 512, :][col_perm, :]

        m = {
            "wq": _pack8(np.ascontiguousarray(wq_g), 8, 512).astype(bf),
            "wk": _pack8(np.ascontiguousarray(wk_g), 8, 128).astype(bf),
            "wv": _pack8(np.ascontiguousarray(wv_g), 8, 128).astype(bf),
            "wout": _pack8(np.ascontiguousarray(wout_g), 4, D_MODEL).astype(bf),
            "cosK": cosK.astype(bf),
            "sinK": sinK.astype(bf),
            "cosQ": cosQ_bf,
            "sinQ": sinQ_bf,
            "maskb": maskb,
            "e2": e2,
        }
        for i, (c0, w) in enumerate(_ktiles(tkv_c)):
            m[f"kv_t{i}"] = _pack8(
                np.ascontiguousarray(kvT[:, c0 : c0 + w]), 8, w
            ).astype(bf)
        for i in range(tq // 1024):
            m[f"q{i}"] = _pack8(
                np.ascontiguousarray(qT[:, i * 1024 : (i + 1) * 1024]), 8, 1024
            ).astype(bf)
        in_maps.append(m)
    return in_maps, tkv_c


_NC_CACHE = {}


T2 = 512


def _get_nc(tq, tkv_c):
    key = (tq, tkv_c, T2)
    if key not in _NC_CACHE:
        _NC_CACHE[key] = build_bass(tq, tkv_c, T2)
    return _NC_CACHE[key]


def _run(inputs, trace=False):
    query = np.asarray(inputs["query"], dtype=np.float32)
    key_value = np.asarray(inputs["key_value"], dtype=np.float32)
    kv_mask = np.asarray(inputs["kv_mask"])
    w_q = np.asarray(inputs["w_q"], dtype=np.float32)
    w_k = np.asarray(inputs["w_k"], dtype=np.float32)
    w_v = np.asarray(inputs["w_v"], dtype=np.float32)
    w_out = np.asarray(inputs["w_out"], dtype=np.float32)
    nb, tq, _ = query.shape

    in_maps, tkv_c = make_in_maps(query, key_value, kv_mask, w_q, w_k, w_v, w_out, tq)
    nc = _get_nc(tq, tkv_c)
    res = run_bass_kernel_spmd(
        nc, in_maps, list(range(2 * nb)), trace=trace, trace_cores=[0]
    )
    outs = [np.asarray(r["out"]) for r in res.results]
    full = np.stack([outs[2 * b] + outs[2 * b + 1] for b in range(nb)])

    query_mask = np.asarray(inputs["query_mask"])
    if not query_mask.all():
        # masked query rows: reference yields uniform attention over all kv
        for b in range(nb):
            rows = ~query_mask[b]
            if rows.any():
                V = key_value[b] @ w_v  # [tkv, 256]
                meanV = V.mean(axis=0)  # [256]
                group = N_HEADS // NUM_KV_HEADS
                feat = np.concatenate([meanV.reshape(NUM_KV_HEADS, D_K)[h // group]
                                       for h in range(N_HEADS)])
                full[b, rows, :] = feat @ w_out
    return full.astype(np.float32), res


def kernel(**inputs):
    out, _ = _run(inputs, trace=False)
    return out


def kernel_traced(**inputs):
    out, res = _run(inputs, trace=True)
    return out, res


if __name__ == "__main__":
    print("kernel.py is a library; use test.py")
